# revision 26
# baseline (speedup 1.0000x reference)
"""Trainium2 Bass kernel for nn_Decay2DBlk (block-decay linear attention).

Full-input contract: kernel(**inputs) takes the unsharded inputs from
setup_inputs() and returns the full [B, T, E] output.

Sharding: 8 cores = 4 batch elements x 2 Dv-halves. Each core computes a
partial output y_b_h = (attn(x_b) restricted to its Dv half) @ Wo[half];
the host sums the two partials per batch element (the "all-reduce after
w_out" done host-side since outputs are gathered anyway).

Math (per core): with d=0.99, D=d^128, block index n, in-block offsets
s,t (keys/queries):
  out[t] = sum_{s<=t} d^(t_g - s_g) * q_t k_s * v_s  @ Wo      (t_g global)
All decay factors are folded into host-precomputed constants so the device
only does matmuls + one elementwise mask multiply + a running-sum state:
  - masks[n][s,t]  = 1[s<=t] d^(-s-1) D^-(n-1)        (intra-block, bf16)
  - vscale[n][t]   = (1-d) d^(127-t) D^-n             (v' for state update)
  - escale[n][t]   = d^(t+1) D^(n-1)                  (final ACT evac scale)
  - state S~ = sum_m k_m^T v'_m  (pure running sum, bf16 in SBUF)
The geometric growth of D^-n keeps every intermediate within fp32/bf16
range (max ~1e19) and makes bf16 rounding of the running sum benign
(validated: rel err ~4e-3 vs fp32 reference).
"""

import os
import sys

for _p in (
    "/root/.axon_site",
    "/root/.axon_site/_ro/trn_rl_repo",
    "/root/.axon_site/_ro/pypackages",
    "/opt/trn_rl_repo",
):
    if os.path.isdir(_p) and _p not in sys.path:
        sys.path.append(_p)

import numpy as np
import ml_dtypes
from contextlib import ExitStack

import concourse.bacc as bacc
import concourse.tile as tile
from concourse import mybir
from concourse.bass_utils import run_bass_kernel_spmd

DECAY = 0.99
TBLK = 128
BF16 = ml_dtypes.bfloat16
E4M3 = ml_dtypes.float8_e4m3fn
BF = mybir.dt.bfloat16
F8 = mybir.dt.float8e4
F32 = mybir.dt.float32
WSCALE = 64.0  # fp8 pre-scale for Wq/Wk (values ~N(0,0.02) -> sweet spot)
DR = mybir.MatmulPerfMode.DoubleRow


def build_nc(T=4096, E=1024, Dk=1024, Dvh=512, ST=512, pair_groups=None):
    """Build the per-core Bass program. Same program runs on all 8 cores
    (SPMD); only the input data differs.

    q/k phase-A work is split across the two cores of a pair: each core
    computes sigmoid(x @ Wq_half + b_half) for the dk-half whose weights it
    was GIVEN as input (wq input is [E, Dk/2]), then the halves are
    exchanged with a pairwise AllGather through DRAM bounce buffers. Core
    2b (group rank 0) always carries the low half, so the gathered layout
    is identical on both cores and the program stays SPMD-symmetric."""
    N = T // TBLK       # number of 128-token blocks
    NU = T // ST        # number of super-tiles
    JB = ST // TBLK     # blocks per super-tile
    EC = E // 128       # E chunks (contraction)
    KC = Dk // 128      # Dk chunks
    HC = KC // 2        # dk chunks computed locally (half)
    DC = Dvh // 128     # Dv-half chunks
    Dkh = Dk // 2
    if pair_groups is None:
        pair_groups = [[0, 1], [2, 3], [4, 5], [6, 7]]

    nc = bacc.Bacc(num_devices=8)
    # all inputs host-pre-arranged to [128-partition, ...contiguous] layout so
    # every DMA is 128 descriptors of 4-8KB (max descriptor efficiency)
    xt = nc.dram_tensor("xt", [NU, 128, EC, ST], BF, kind="ExternalInput")
    xt8 = nc.dram_tensor("xt8", [NU, 128, EC, ST], F8, kind="ExternalInput")
    wq = nc.dram_tensor("wq", [128, EC, Dkh], F8, kind="ExternalInput")
    wk = nc.dram_tensor("wk", [128, EC, Dkh], F8, kind="ExternalInput")
    wv = nc.dram_tensor("wv", [128, EC, Dvh], BF, kind="ExternalInput")
    wo = nc.dram_tensor("wo", [128, DC, E], BF, kind="ExternalInput")
    bq = nc.dram_tensor("bq", [128, HC], F32, kind="ExternalInput")
    bk = nc.dram_tensor("bk", [128, HC], F32, kind="ExternalInput")
    masks = nc.dram_tensor("masks", [128, N, 128], BF, kind="ExternalInput")
    vscale = nc.dram_tensor("vscale", [128, N], F32, kind="ExternalInput")
    escale = nc.dram_tensor("escale", [128, N], F32, kind="ExternalInput")
    ident = nc.dram_tensor("ident", [128, 128], F8, kind="ExternalInput")
    out = nc.dram_tensor("out", [T, E], BF, kind="ExternalOutput")

    SIG = mybir.ActivationFunctionType.Sigmoid
    CPY = mybir.ActivationFunctionType.Copy

    with tile.TileContext(nc) as tc:
        with ExitStack() as ctx:
            consts = ctx.enter_context(tc.tile_pool(name="consts", bufs=1))
            qkh_pool = ctx.enter_context(tc.tile_pool(name="qkh", bufs=2))
            dram = ctx.enter_context(tc.tile_pool(name="dram", bufs=2, space="DRAM"))
            xt_pool = ctx.enter_context(tc.tile_pool(name="xt", bufs=3))
            xt8_pool = ctx.enter_context(tc.tile_pool(name="xt8", bufs=3))
            qk_pool = ctx.enter_context(tc.tile_pool(name="qk", bufs=3))
            v_pool = ctx.enter_context(tc.tile_pool(name="v", bufs=3))
            kn_pool = ctx.enter_context(tc.tile_pool(name="kn", bufs=3))
            ap_pool = ctx.enter_context(tc.tile_pool(name="apool", bufs=3))
            yt_pool = ctx.enter_context(tc.tile_pool(name="yt", bufs=3))
            o_pool = ctx.enter_context(tc.tile_pool(name="opool", bufs=2))
            st_pool = ctx.enter_context(tc.tile_pool(name="state", bufs=1))
            psA = ctx.enter_context(tc.tile_pool(name="psA", bufs=2, space="PSUM"))
            psB = ctx.enter_context(tc.tile_pool(name="psB", bufs=3, space="PSUM"))
            psY = ctx.enter_context(tc.tile_pool(name="psY", bufs=2, space="PSUM"))
            psa = ctx.enter_context(tc.tile_pool(name="psa", bufs=1, space="PSUM"))

            # ---- constants into SBUF ----
            # startup-latency critical: the very first matmul needs only
            # wq8 + bq + xt8[0] (1.25MB total in fp8) -- issue those first;
            # everything else streams in under cover of the fp8 q/k matmuls
            wq_sb = consts.tile([128, EC, Dkh], F8)
            nc.sync.dma_start(out=wq_sb, in_=wq[:, :, :])
            bq_sb = consts.tile([128, HC], F32)
            nc.sync.dma_start(out=bq_sb, in_=bq[:, :])
            wk_sb = consts.tile([128, EC, Dkh], F8)
            nc.sync.dma_start(out=wk_sb, in_=wk[:, :, :])
            bk_sb = consts.tile([128, HC], F32)
            nc.sync.dma_start(out=bk_sb, in_=bk[:, :])
            # deferred const tiles (DMAs issued inside/after phase A 0)
            vs_sb = consts.tile([128, N], F32)
            es_sb = consts.tile([128, N], F32)
            id_sb = consts.tile([128, 128], F8)
            wv_sb = consts.tile([128, EC, Dvh], BF)
            mk_sb = consts.tile([128, N, 128], BF)
            wo_sb = consts.tile([128, DC, E], BF)

            def emit_deferred_consts():
                nc.sync.dma_start(out=wv_sb, in_=wv[:, :, :])
                nc.sync.dma_start(out=vs_sb, in_=vscale[:, :])
                nc.sync.dma_start(out=es_sb, in_=escale[:, :])
                nc.sync.dma_start(out=id_sb, in_=ident[:, :])

            def emit_warm_collective(junk):
                # tiny dummy AllGather at t~0: absorbs the one-time ~30us
                # collective rendezvous/setup latency while phase A runs
                win = dram.tile([128, 16], BF, name="wagi", tag="wagi")
                nc.scalar.dma_start(out=win, in_=junk[:, :16])
                wout = dram.tile([2, 128, 16], BF, name="wago", tag="wago")
                nc.gpsimd.collective_compute(
                    "AllGather", mybir.AluOpType.bypass,
                    replica_groups=pair_groups,
                    ins=[win.opt()], outs=[wout.opt()])

            # persistent scaled-sum state S~ [dk, dv], one tile per dk-chunk
            # (separate tiles -> per-chunk dependency chains, so the y2 reads
            # of block n+1 pipeline against the state adds of block n)
            S_c = [st_pool.tile([128, Dvh], BF, name=f"S{c}", tag=f"S{c}")
                   for c in range(KC)]

            def emit_phase_a(u, first=False):
                # ---- load xT super-tile: fp8 copy first (feeds q/k
                # DoubleRow matmuls immediately), bf16 behind it (for v) ----
                xt8_u = xt8_pool.tile([128, EC, ST], F8, name="xt8_u")
                nc.sync.dma_start(out=xt8_u, in_=xt8[u])
                xt_u = xt_pool.tile([128, EC, ST], BF, name="xt_u")
                nc.sync.dma_start(out=xt_u, in_=xt[u])
                if first:
                    emit_deferred_consts()

                # ---- phase A: this core computes its dk-half of qT, kT ----
                # fp8 DoubleRow: contraction pairs of e-chunks, 2x PE rate.
                # q and k halves live in ONE tile [128, 2(qk), HC, ST] so the
                # exchange staging is a single DMA trigger.
                qkh_u = qkh_pool.tile([128, 2, HC, ST], F8, name="qkh_u", tag="qkh")
                for qk, (w_sb, b_sb) in enumerate(((wq_sb, bq_sb), (wk_sb, bk_sb))):
                    for c in range(HC):
                        ps = psA.tile([128, ST], F32, name="psqk", tag="psA")
                        for e in range(EC // 2):
                            nc.tensor.matmul(
                                ps, w_sb[:, 2 * e:2 * e + 2, c * 128:(c + 1) * 128],
                                xt8_u[:, 2 * e:2 * e + 2, :],
                                start=(e == 0), stop=(e == EC // 2 - 1), perf_mode=DR)
                        nc.scalar.activation(qkh_u[:, qk, c, :], ps, SIG,
                                             bias=b_sb[:, c:c + 1], scale=1.0 / WSCALE)

                v_u = v_pool.tile([128, JB, Dvh], BF, name="v_u", tag="v")
                for j in range(JB):
                    ps = psA.tile([128, ST], F32, name="psv", tag="psA")[:, :Dvh]
                    for e in range(EC):
                        nc.tensor.matmul(
                            ps, xt_u[:, e, j * 128:(j + 1) * 128], wv_sb[:, e, :],
                            start=(e == 0), stop=(e == EC - 1))
                    nc.scalar.activation(v_u[:, j, :], ps, CPY, scale=1.0 - DECAY)
                return qkh_u, v_u

            def emit_exchange(u, ph):
                # pairwise AllGather of the q/k dk-halves through DRAM bounce
                # buffers; staging DMAs ride the sync HWDGE ring (fast static
                # descriptors) -- only the collective itself is on CC
                qkh_u, _ = ph
                bin_u = dram.tile([128, 2, HC, ST], F8, name="bin_u", tag="bin")
                nc.sync.dma_start(out=bin_u, in_=qkh_u)
                bout_u = dram.tile([2, 128, 2, HC, ST], F8, name="bout_u", tag="bout")
                nc.gpsimd.collective_compute(
                    "AllGather", mybir.AluOpType.bypass,
                    replica_groups=pair_groups,
                    ins=[bin_u.opt()], outs=[bout_u.opt()])
                return bout_u

            def emit_gather(u, bout_u):
                # one DMA: [g, p, qk, h, t] -> qk_u[p, qk, g*HC+h, t]
                qk_u = qk_pool.tile([128, 2, KC, ST], F8, name="qk_u", tag="qk")
                nc.sync.dma_start(
                    out=qk_u, in_=bout_u.rearrange("g p q h t -> p q g h t"))
                return qk_u

            def emit_blocks(u, tiles, qk_u):
                _, v_u = tiles
                qT_u = qk_u[:, 0]
                kT_u = qk_u[:, 1]
                # ---- block loop ----
                for j in range(JB):
                    n = u * JB + j
                    jsl = slice(j * 128, (j + 1) * 128)

                    # intra-block attention logits a^T[s,t], masked
                    a_ps = psa.tile([128, 128], F32, name="a_ps")
                    for c in range(KC):
                        nc.tensor.matmul(
                            a_ps, kT_u[:, c, jsl], qT_u[:, c, jsl],
                            start=(c == 0), stop=(c == KC - 1))
                    a_sb = ap_pool.tile([128, 128], BF, name="a_sb")
                    nc.vector.tensor_mul(a_sb, a_ps, mk_sb[:, n, :])

                    # k natural [s, dk] via PE transposes of kT, scaled by
                    # vscale[s,n] during the PSUM->SBUF copy (so the kv state
                    # matmul can consume plain v_u -- no separate vp tensor)
                    kn = kn_pool.tile([128, Dk], BF, name="kn")
                    for h in range(KC // 4):
                        # fp8 PE transpose writes with element step 2 (hw
                        # requirement), so each 128-col result spans 256
                        # elements; the strided AP is collapsed by the copy
                        tp = psB.tile([128, 1024], F8, name="tp", tag="psB")[:, 0:1024:2]
                        for q4 in range(4):
                            c = h * 4 + q4
                            nc.tensor.transpose(
                                tp[:, q4 * 128:(q4 + 1) * 128],
                                kT_u[:, c, jsl], id_sb)
                        if h == 0:
                            nc.scalar.activation(
                                kn[:, h * 512:(h + 1) * 512], tp, CPY,
                                scale=vs_sb[:, n:n + 1])
                        else:
                            nc.vector.tensor_scalar_mul(
                                kn[:, h * 512:(h + 1) * 512], tp,
                                vs_sb[:, n:n + 1])

                    # yT[dv, t] = v^T a' + S~^T-contract (cross), unscaled
                    y_ps = psY.tile([128, 4 * 128], F32, name="y_ps")
                    for dc in range(DC):
                        osl = slice(dc * 128, (dc + 1) * 128)
                        dvsl = slice(dc * 128, (dc + 1) * 128)
                        nc.tensor.matmul(
                            y_ps[:, osl], v_u[:, j, dvsl], a_sb,
                            start=True, stop=(n == 0))
                        if n > 0:
                            for c in range(KC):
                                nc.tensor.matmul(
                                    y_ps[:, osl], S_c[c][:, dvsl], qT_u[:, c, jsl],
                                    start=False, stop=(c == KC - 1))
                    yT_sb = yt_pool.tile([128, 4 * 128], BF, name="yT_sb")
                    nc.scalar.copy(yT_sb, y_ps)

                    # state update S~ += k'^T v (DVE adds trail the kv
                    # matmuls) interleaved with the out projection so the
                    # PSUM bank recycling (freed by each DVE add) always has
                    # a few hundred ns of PE work in front of it
                    o_sb = o_pool.tile([128, E], BF, name="o_sb")

                    def emit_kv(c):
                        kv_ps = psB.tile([128, 512], F32, name="kv_ps", tag="psB")[:, :Dvh]
                        nc.tensor.matmul(
                            kv_ps, kn[:, c * 128:(c + 1) * 128], v_u[:, j, :],
                            start=True, stop=True)
                        if n == 0:
                            nc.vector.tensor_copy(S_c[c], kv_ps)
                        else:
                            nc.vector.tensor_add(S_c[c], S_c[c], kv_ps)

                    def emit_out(hh):
                        # out[t, e-half] = yT^T @ Wo, evacuated with escale
                        o_ps = psB.tile([128, 512], F32, name="o_ps", tag="psB")
                        for dc in range(DC):
                            nc.tensor.matmul(
                                o_ps, yT_sb[:, dc * 128:(dc + 1) * 128],
                                wo_sb[:, dc, hh * 512:(hh + 1) * 512],
                                start=(dc == 0), stop=(dc == DC - 1))
                        nc.scalar.activation(
                            o_sb[:, hh * 512:(hh + 1) * 512], o_ps, CPY,
                            scale=es_sb[:, n:n + 1])

                    for c in range(4):
                        emit_kv(c)
                    emit_out(0)
                    for c in range(4, KC):
                        emit_kv(c)
                    emit_out(1)
                    # out store rides the scalar HWDGE ring: its dependency
                    # (the o_sb evac, also on scalar) is met by queue order,
                    # so it never head-of-line-blocks anything
                    nc.scalar.dma_start(
                        out=out[n * 128:(n + 1) * 128, :], in_=o_sb)

            # ---- PE warmup: dummy matmuls on never-written SBUF keep the
            # tensor engine busy from t~0 so the HAM clock-gate reaches
            # K=8/8 (2.4GHz) before the first real matmul, instead of
            # running phase A at 1.2GHz ----
            junk = consts.tile([128, 128], BF, name="junk")
            nc.vector.memset(junk, 0)
            emit_warm_collective(junk)
            wps = psB.tile([128, 512], F32, name="warm", tag="psB")
            for _ in range(44):
                nc.tensor.matmul(wps[:, :128], junk, junk, start=True, stop=True)

            # Software pipeline, depth 3: phase A for u+3 and the exchange
            # for u+1 are issued before the block loop of u; the gather of
            # u lands just ahead of its block loop.
            pend = {0: emit_phase_a(0, first=True)}
            for uu in (1, 2):
                if uu < NU:
                    pend[uu] = emit_phase_a(uu)
            xch = {0: emit_exchange(0, pend[0])}
            nc.sync.dma_start(out=mk_sb, in_=masks[:, :, :])
            nc.sync.dma_start(out=wo_sb, in_=wo[:, :, :])
            for u in range(NU):
                qkt = emit_gather(u, xch.pop(u))
                if u + 3 < NU:
                    pend[u + 3] = emit_phase_a(u + 3)
                if u + 1 < NU:
                    xch[u + 1] = emit_exchange(u + 1, pend[u + 1])
                emit_blocks(u, pend.pop(u), qkt)
    return nc


def make_host_constants(T=4096, dtype_np=np.float32):
    """Host-precomputed decay constants (see module docstring)."""
    N = T // TBLK
    d = np.float64(DECAY)
    D128 = d ** TBLK
    s = np.arange(TBLK, dtype=np.float64)
    t = np.arange(TBLK, dtype=np.float64)
    nn = np.arange(N, dtype=np.float64)

    # masks[s, n, t] = 1[s<=t] * d^(-s-1) * D128^-(n-1)
    tri = (s[:, None] <= t[None, :]).astype(np.float64)  # [s, t]
    m = tri[:, None, :] * (d ** (-s - 1.0))[:, None, None] \
        * (D128 ** (-(nn - 1.0)))[None, :, None]
    masks = m.astype(BF16)

    # vscale[t, n] = d^(127-t) D128^-n  (applied to kn against v_u, which
    # already carries the (1-d) factor)
    vsc = (d ** (127.0 - t))[:, None] * (D128 ** (-nn))[None, :]
    vscale = vsc.astype(np.float32)

    # escale[t, n] = d^(t+1) D128^(n-1)
    esc = (d ** (t + 1.0))[:, None] * (D128 ** (nn - 1.0))[None, :]
    escale = esc.astype(np.float32)

    ident = np.eye(128, dtype=E4M3)
    return masks, vscale, escale, ident


_NC_CACHE = {}


def _get_nc(T, E, Dk, Dvh):
    key = (T, E, Dk, Dvh)
    if key not in _NC_CACHE:
        nc = build_nc(T=T, E=E, Dk=Dk, Dvh=Dvh)
        nc.finalize()
        _NC_CACHE[key] = nc
    return _NC_CACHE[key]


def kernel(x, Wv, Wk, bk, Wq, bq, Wo):
    y, _ = run(x, Wv, Wk, bk, Wq, bq, Wo)
    return y


def _install_ntff_hook():
    """The agent image's antenv lacks axon_hooks; recreate it from
    trn_boot's ctypes NTFF driver so trace=True produces profiles."""
    try:
        from antenv.axon_hooks import get_axon_ntff_profile_hook  # noqa: F401
        return
    except ImportError:
        pass
    try:
        import types
        import antenv
        from trn_agent_boot.trn_boot import _ntff_profile_via_ctypes
        hook = _ntff_profile_via_ctypes("/opt/axon/libaxon_pjrt.so")
        mod = types.ModuleType("antenv.axon_hooks")
        _h = {"hook": hook}
        mod.get_axon_ntff_profile_hook = lambda: _h["hook"]
        mod.set_axon_ntff_profile_hook = lambda h: _h.update(hook=h)
        sys.modules["antenv.axon_hooks"] = mod
        antenv.axon_hooks = mod
    except Exception as e:  # profiling is best-effort
        print(f"ntff hook install failed: {e}")


def _arrange_xt(xb, ST=512, dtype=BF16):
    """x[b] [T, E] -> xT pre-tiled [NU, 128, EC, ST], contiguous."""
    T, E = xb.shape
    xT = np.ascontiguousarray(xb.T).astype(dtype)         # [E, T]
    EC, NU = E // 128, T // ST
    return np.ascontiguousarray(
        xT.reshape(EC, 128, NU, ST).transpose(2, 1, 0, 3))


def _q8(a, scale=1.0):
    """Quantize to fp8 e4m3 (TRN-compatible range: clip +-240)."""
    return np.clip(np.asarray(a, np.float32) * scale, -240.0, 240.0).astype(E4M3)


def _arrange_w(w):
    """[E-or-Dv, D] -> [128, chunks, D] with row = chunk*128 + p."""
    R, D = w.shape
    C = R // 128
    return np.ascontiguousarray(w.reshape(C, 128, D).transpose(1, 0, 2))


def _arrange_b(b):
    b = np.asarray(b, np.float32).reshape(-1)
    C = b.shape[0] // 128
    return np.ascontiguousarray(b.reshape(C, 128).T)


def run(x, Wv, Wk, bk, Wq, bq, Wo, trace=False):
    x = np.asarray(x)
    B, T, E = x.shape
    Dk = np.asarray(Wk).shape[1]
    Dv = np.asarray(Wv).shape[1]
    Dvh = Dv // 2
    assert B == 4, "sharding is hardcoded for B=4 x 2 Dv-halves"

    nc = _get_nc(T, E, Dk, Dvh)
    masks, vscale, escale, ident = make_host_constants(T=T)

    wq_f8 = _q8(Wq, WSCALE)
    wk_f8 = _q8(Wk, WSCALE)
    bq32 = np.asarray(bq, np.float32).reshape(Dk, 1)
    bk32 = np.asarray(bk, np.float32).reshape(Dk, 1)
    Dkh = Dk // 2

    xt_cache = [_arrange_xt(x[b]) for b in range(B)]
    xt8_cache = [_arrange_xt(x[b], dtype=E4M3) for b in range(B)]
    in_maps = []
    for c in range(8):
        b, h = divmod(c, 2)
        dvs = slice(h * Dvh, (h + 1) * Dvh)
        # this core computes the q/k dk-half matching its pair rank
        dks = slice(h * Dkh, (h + 1) * Dkh)
        in_maps.append({
            "xt": xt_cache[b],
            "xt8": xt8_cache[b],
            "wq": _arrange_w(wq_f8[:, dks]),
            "wk": _arrange_w(wk_f8[:, dks]),
            "wv": _arrange_w(np.asarray(Wv[:, dvs], BF16)),
            "wo": _arrange_w(np.asarray(Wo[dvs], BF16)),
            "bq": _arrange_b(bq32[dks]),
            "bk": _arrange_b(bk32[dks]),
            "masks": masks,
            "vscale": vscale,
            "escale": escale,
            "ident": ident,
        })

    if trace:
        _install_ntff_hook()
    res = run_bass_kernel_spmd(nc, in_maps, core_ids=list(range(8)), trace=trace)
    y = np.zeros((B, T, E), np.float32)
    for c in range(8):
        b = c // 2
        y[b] += np.asarray(res.results[c]["out"], np.float32)
    return y, res



# revision 27
# speedup vs baseline: 1.1766x; 1.1766x over previous
"""Trainium2 Bass kernel for nn_Decay2DBlk (block-decay linear attention).

Full-input contract: kernel(**inputs) takes the unsharded inputs from
setup_inputs() and returns the full [B, T, E] output.

Sharding: 8 cores = 4 batch elements x 2 Dv-halves. Each core computes a
partial output y_b_h = (attn(x_b) restricted to its Dv half) @ Wo[half];
the host sums the two partials per batch element (the "all-reduce after
w_out" done host-side since outputs are gathered anyway).

Math (per core): with d=0.99, D=d^128, block index n, in-block offsets
s,t (keys/queries):
  out[t] = sum_{s<=t} d^(t_g - s_g) * q_t k_s * v_s  @ Wo      (t_g global)
All decay factors are folded into host-precomputed constants so the device
only does matmuls + one elementwise mask multiply + a running-sum state:
  - masks[n][s,t]  = 1[s<=t] d^(-s-1) D^-(n-1)        (intra-block, bf16)
  - vscale[n][t]   = (1-d) d^(127-t) D^-n             (v' for state update)
  - escale[n][t]   = d^(t+1) D^(n-1)                  (final ACT evac scale)
  - state S~ = sum_m k_m^T v'_m  (pure running sum, bf16 in SBUF)
The geometric growth of D^-n keeps every intermediate within fp32/bf16
range (max ~1e19) and makes bf16 rounding of the running sum benign
(validated: rel err ~4e-3 vs fp32 reference).
"""

import os
import sys

for _p in (
    "/root/.axon_site",
    "/root/.axon_site/_ro/trn_rl_repo",
    "/root/.axon_site/_ro/pypackages",
    "/opt/trn_rl_repo",
):
    if os.path.isdir(_p) and _p not in sys.path:
        sys.path.append(_p)

import numpy as np
import ml_dtypes
from contextlib import ExitStack

import concourse.bacc as bacc
import concourse.tile as tile
from concourse import mybir
from concourse.bass_utils import run_bass_kernel_spmd

DECAY = 0.99
TBLK = 128
BF16 = ml_dtypes.bfloat16
E4M3 = ml_dtypes.float8_e4m3fn
BF = mybir.dt.bfloat16
F8 = mybir.dt.float8e4
F32 = mybir.dt.float32
WSCALE = 64.0  # fp8 pre-scale for Wq/Wk (values ~N(0,0.02) -> sweet spot)
DR = mybir.MatmulPerfMode.DoubleRow


def build_nc(T=4096, E=1024, Dk=1024, Dvh=512, ST=512, pair_groups=None):
    """Build the per-core Bass program. Same program runs on all 8 cores
    (SPMD); only the input data differs.

    q/k phase-A work is split across the two cores of a pair: each core
    computes sigmoid(x @ Wq_half + b_half) for the dk-half whose weights it
    was GIVEN as input (wq input is [E, Dk/2]), then the halves are
    exchanged with a pairwise AllGather through DRAM bounce buffers. Core
    2b (group rank 0) always carries the low half, so the gathered layout
    is identical on both cores and the program stays SPMD-symmetric."""
    N = T // TBLK       # number of 128-token blocks
    NU = T // ST        # number of super-tiles
    JB = ST // TBLK     # blocks per super-tile
    EC = E // 128       # E chunks (contraction)
    KC = Dk // 128      # Dk chunks
    HC = KC // 2        # dk chunks computed locally (half)
    DC = Dvh // 128     # Dv-half chunks
    Dkh = Dk // 2
    if pair_groups is None:
        pair_groups = [[0, 1], [2, 3], [4, 5], [6, 7]]

    nc = bacc.Bacc(num_devices=8)
    # all inputs host-pre-arranged to [128-partition, ...contiguous] layout so
    # every DMA is 128 descriptors of 4-8KB (max descriptor efficiency)
    xt = nc.dram_tensor("xt", [NU, 128, EC, ST], BF, kind="ExternalInput")
    xt8 = nc.dram_tensor("xt8", [NU, 128, EC, ST], F8, kind="ExternalInput")
    wq = nc.dram_tensor("wq", [128, EC, Dkh], F8, kind="ExternalInput")
    wk = nc.dram_tensor("wk", [128, EC, Dkh], F8, kind="ExternalInput")
    wv = nc.dram_tensor("wv", [128, EC, Dvh], BF, kind="ExternalInput")
    wo = nc.dram_tensor("wo", [128, DC, E], BF, kind="ExternalInput")
    bq = nc.dram_tensor("bq", [128, HC], F32, kind="ExternalInput")
    bk = nc.dram_tensor("bk", [128, HC], F32, kind="ExternalInput")
    masks = nc.dram_tensor("masks", [128, N, 128], BF, kind="ExternalInput")
    vscale = nc.dram_tensor("vscale", [128, N], F32, kind="ExternalInput")
    escale = nc.dram_tensor("escale", [128, N], F32, kind="ExternalInput")
    ident = nc.dram_tensor("ident", [128, 128], F8, kind="ExternalInput")
    out = nc.dram_tensor("out", [T, E], BF, kind="ExternalOutput")

    SIG = mybir.ActivationFunctionType.Sigmoid
    CPY = mybir.ActivationFunctionType.Copy

    with tile.TileContext(nc) as tc:
        with ExitStack() as ctx:
            consts = ctx.enter_context(tc.tile_pool(name="consts", bufs=1))
            qkh_pool = ctx.enter_context(tc.tile_pool(name="qkh", bufs=2))
            dram = ctx.enter_context(tc.tile_pool(name="dram", bufs=6, space="DRAM"))
            xt_pool = ctx.enter_context(tc.tile_pool(name="xt", bufs=3))
            xt8_pool = ctx.enter_context(tc.tile_pool(name="xt8", bufs=3))
            qk_pool = ctx.enter_context(tc.tile_pool(name="qk", bufs=3))
            v_pool = ctx.enter_context(tc.tile_pool(name="v", bufs=6))
            kn_pool = ctx.enter_context(tc.tile_pool(name="kn", bufs=3))
            ap_pool = ctx.enter_context(tc.tile_pool(name="apool", bufs=3))
            yt_pool = ctx.enter_context(tc.tile_pool(name="yt", bufs=3))
            o_pool = ctx.enter_context(tc.tile_pool(name="opool", bufs=2))
            st_pool = ctx.enter_context(tc.tile_pool(name="state", bufs=1))
            psA = ctx.enter_context(tc.tile_pool(name="psA", bufs=2, space="PSUM"))
            psB = ctx.enter_context(tc.tile_pool(name="psB", bufs=3, space="PSUM"))
            psY = ctx.enter_context(tc.tile_pool(name="psY", bufs=2, space="PSUM"))
            psa = ctx.enter_context(tc.tile_pool(name="psa", bufs=1, space="PSUM"))

            # ---- constants into SBUF ----
            # startup-latency critical: the very first matmul needs only
            # wq8 + bq + xt8[0] (1.25MB total in fp8) -- issue those first;
            # everything else streams in under cover of the fp8 q/k matmuls
            wq_sb = consts.tile([128, EC, Dkh], F8)
            nc.sync.dma_start(out=wq_sb, in_=wq[:, :, :])
            bq_sb = consts.tile([128, HC], F32)
            nc.sync.dma_start(out=bq_sb, in_=bq[:, :])
            wk_sb = consts.tile([128, EC, Dkh], F8)
            nc.sync.dma_start(out=wk_sb, in_=wk[:, :, :])
            bk_sb = consts.tile([128, HC], F32)
            nc.sync.dma_start(out=bk_sb, in_=bk[:, :])
            # deferred const tiles (DMAs issued inside/after phase A 0)
            vs_sb = consts.tile([128, N], F32)
            es_sb = consts.tile([128, N], F32)
            id_sb = consts.tile([128, 128], F8)
            wv_sb = consts.tile([128, EC, Dvh], BF)
            mk_sb = consts.tile([128, N, 128], BF)
            wo_sb = consts.tile([128, DC, E], BF)

            def emit_deferred_consts():
                nc.sync.dma_start(out=wv_sb, in_=wv[:, :, :])
                nc.sync.dma_start(out=vs_sb, in_=vscale[:, :])
                nc.sync.dma_start(out=es_sb, in_=escale[:, :])
                nc.sync.dma_start(out=id_sb, in_=ident[:, :])

            # persistent scaled-sum state S~ [dk, dv], one tile per dk-chunk
            # (separate tiles -> per-chunk dependency chains, so the y2 reads
            # of block n+1 pipeline against the state adds of block n)
            S_c = [st_pool.tile([128, Dvh], BF, name=f"S{c}", tag=f"S{c}")
                   for c in range(KC)]

            def emit_phase_a(u, first=False):
                # ---- load xT super-tile: fp8 copy first (feeds q/k
                # DoubleRow matmuls immediately), bf16 behind it (for v) ----
                xt8_u = xt8_pool.tile([128, EC, ST], F8, name="xt8_u")
                nc.sync.dma_start(out=xt8_u, in_=xt8[u])
                xt_u = xt_pool.tile([128, EC, ST], BF, name="xt_u")
                nc.sync.dma_start(out=xt_u, in_=xt[u])
                if first:
                    emit_deferred_consts()

                # ---- phase A: this core computes its dk-half of qT, kT ----
                # fp8 DoubleRow: contraction pairs of e-chunks, 2x PE rate.
                # q and k halves live in ONE tile [128, 2(qk), HC, ST] so the
                # exchange staging is a single DMA trigger.
                qkh_u = qkh_pool.tile([128, 2, HC, ST], F8, name="qkh_u", tag="qkh")
                for qk, (w_sb, b_sb) in enumerate(((wq_sb, bq_sb), (wk_sb, bk_sb))):
                    for c in range(HC):
                        ps = psA.tile([128, ST], F32, name="psqk", tag="psA")
                        for e in range(EC // 2):
                            nc.tensor.matmul(
                                ps, w_sb[:, 2 * e:2 * e + 2, c * 128:(c + 1) * 128],
                                xt8_u[:, 2 * e:2 * e + 2, :],
                                start=(e == 0), stop=(e == EC // 2 - 1), perf_mode=DR)
                        nc.scalar.activation(qkh_u[:, qk, c, :], ps, SIG,
                                             bias=b_sb[:, c:c + 1], scale=1.0 / WSCALE)

                v_u = v_pool.tile([128, JB, Dvh], BF, name="v_u", tag="v")
                for j in range(JB):
                    ps = psA.tile([128, ST], F32, name="psv", tag="psA")[:, :Dvh]
                    for e in range(EC):
                        nc.tensor.matmul(
                            ps, xt_u[:, e, j * 128:(j + 1) * 128], wv_sb[:, e, :],
                            start=(e == 0), stop=(e == EC - 1))
                    nc.scalar.activation(v_u[:, j, :], ps, CPY, scale=1.0 - DECAY)
                return qkh_u, v_u

            def emit_exchange(u, ph):
                # pairwise AllGather of the q/k dk-halves through DRAM bounce
                # buffers; staging DMAs ride the sync HWDGE ring (fast static
                # descriptors) -- only the collective itself is on CC
                qkh_u, _ = ph
                bin_u = dram.tile([128, 2, HC, ST], F8, name="bin_u", tag="bin")
                nc.scalar.dma_start(out=bin_u, in_=qkh_u)
                bout_u = dram.tile([2, 128, 2, HC, ST], F8, name="bout_u", tag="bout")
                nc.gpsimd.collective_compute(
                    "AllGather", mybir.AluOpType.bypass,
                    replica_groups=pair_groups,
                    ins=[bin_u.opt()], outs=[bout_u.opt()])
                return bout_u

            def emit_gather(u, bout_u):
                # one DMA: [g, p, qk, h, t] -> qk_u[p, qk, g*HC+h, t]
                qk_u = qk_pool.tile([128, 2, KC, ST], F8, name="qk_u", tag="qk")
                nc.gpsimd.dma_start(
                    out=qk_u, in_=bout_u.rearrange("g p q h t -> p q g h t"))
                return qk_u

            def emit_blocks(u, tiles, qk_u):
                _, v_u = tiles
                qT_u = qk_u[:, 0]
                kT_u = qk_u[:, 1]
                # ---- block loop ----
                for j in range(JB):
                    n = u * JB + j
                    jsl = slice(j * 128, (j + 1) * 128)

                    # intra-block attention logits a^T[s,t], masked
                    a_ps = psa.tile([128, 128], F32, name="a_ps")
                    for c in range(KC):
                        nc.tensor.matmul(
                            a_ps, kT_u[:, c, jsl], qT_u[:, c, jsl],
                            start=(c == 0), stop=(c == KC - 1))
                    a_sb = ap_pool.tile([128, 128], BF, name="a_sb")
                    nc.vector.tensor_mul(a_sb, a_ps, mk_sb[:, n, :])

                    # k natural [s, dk] via PE transposes of kT, scaled by
                    # vscale[s,n] during the PSUM->SBUF copy (so the kv state
                    # matmul can consume plain v_u -- no separate vp tensor)
                    kn = kn_pool.tile([128, Dk], BF, name="kn")
                    for h in range(KC // 4):
                        # fp8 PE transpose writes with element step 2 (hw
                        # requirement), so each 128-col result spans 256
                        # elements; the strided AP is collapsed by the copy
                        tp = psB.tile([128, 1024], F8, name="tp", tag="psB")[:, 0:1024:2]
                        for q4 in range(4):
                            c = h * 4 + q4
                            nc.tensor.transpose(
                                tp[:, q4 * 128:(q4 + 1) * 128],
                                kT_u[:, c, jsl], id_sb)
                        if h == 0:
                            nc.scalar.activation(
                                kn[:, h * 512:(h + 1) * 512], tp, CPY,
                                scale=vs_sb[:, n:n + 1])
                        else:
                            nc.scalar.activation(
                                kn[:, h * 512:(h + 1) * 512], tp, CPY,
                                scale=vs_sb[:, n:n + 1])

                    # yT[dv, t] = v^T a' + S~^T-contract (cross), unscaled
                    y_ps = psY.tile([128, 4 * 128], F32, name="y_ps")
                    for dc in range(DC):
                        osl = slice(dc * 128, (dc + 1) * 128)
                        dvsl = slice(dc * 128, (dc + 1) * 128)
                        nc.tensor.matmul(
                            y_ps[:, osl], v_u[:, j, dvsl], a_sb,
                            start=True, stop=(n == 0))
                        if n > 0:
                            for c in range(KC):
                                nc.tensor.matmul(
                                    y_ps[:, osl], S_c[c][:, dvsl], qT_u[:, c, jsl],
                                    start=False, stop=(c == KC - 1))
                    yT_sb = yt_pool.tile([128, 4 * 128], BF, name="yT_sb")
                    nc.scalar.copy(yT_sb, y_ps)

                    # state update S~ += k'^T v (DVE adds trail the kv
                    # matmuls) interleaved with the out projection so the
                    # PSUM bank recycling (freed by each DVE add) always has
                    # a few hundred ns of PE work in front of it
                    o_sb = o_pool.tile([128, E], BF, name="o_sb")

                    def emit_kv(c):
                        kv_ps = psB.tile([128, 512], F32, name="kv_ps", tag="psB")[:, :Dvh]
                        nc.tensor.matmul(
                            kv_ps, kn[:, c * 128:(c + 1) * 128], v_u[:, j, :],
                            start=True, stop=True)
                        if n == 0:
                            nc.vector.tensor_copy(S_c[c], kv_ps)
                        else:
                            nc.vector.tensor_add(S_c[c], S_c[c], kv_ps)

                    def emit_out(hh):
                        # out[t, e-half] = yT^T @ Wo, evacuated with escale
                        o_ps = psB.tile([128, 512], F32, name="o_ps", tag="psB")
                        for dc in range(DC):
                            nc.tensor.matmul(
                                o_ps, yT_sb[:, dc * 128:(dc + 1) * 128],
                                wo_sb[:, dc, hh * 512:(hh + 1) * 512],
                                start=(dc == 0), stop=(dc == DC - 1))
                        nc.scalar.activation(
                            o_sb[:, hh * 512:(hh + 1) * 512], o_ps, CPY,
                            scale=es_sb[:, n:n + 1])

                    for c in range(4):
                        emit_kv(c)
                    emit_out(0)
                    for c in range(4, KC):
                        emit_kv(c)
                    emit_out(1)
                    # out store rides the scalar HWDGE ring: its dependency
                    # (the o_sb evac, also on scalar) is met by queue order,
                    # so it never head-of-line-blocks anything
                    nc.scalar.dma_start(
                        out=out[n * 128:(n + 1) * 128, :], in_=o_sb)

            # ---- PE warmup: dummy matmuls on never-written SBUF keep the
            # tensor engine busy from t~0 so the HAM clock-gate reaches
            # K=8/8 (2.4GHz) before the first real matmul, instead of
            # running phase A at 1.2GHz ----
            junk = consts.tile([128, 128], BF, name="junk")
            nc.vector.memset(junk, 0)
            wps = psB.tile([128, 512], F32, name="warm", tag="psB")
            for _ in range(44):
                nc.tensor.matmul(wps[:, :128], junk, junk, start=True, stop=True)

            # Software pipeline, depth 5: the AllGather control plane has
            # a ~40us latency floor per call, so the first 5 phase A's are
            # issued up front -- the PE chews through them while the first
            # exchanges complete, and every gather lands with slack.
            DEPTH = 5
            pend = {}
            xch = {}
            for uu in range(min(DEPTH, NU)):
                pend[uu] = emit_phase_a(uu, first=(uu == 0))
                xch[uu] = emit_exchange(uu, pend[uu])
                if uu == 2:
                    nc.sync.dma_start(out=mk_sb, in_=masks[:, :, :])
                    nc.sync.dma_start(out=wo_sb, in_=wo[:, :, :])
            for u in range(NU):
                qkt = emit_gather(u, xch.pop(u))
                emit_blocks(u, pend.pop(u), qkt)
                if u + DEPTH < NU:
                    pend[u + DEPTH] = emit_phase_a(u + DEPTH)
                    xch[u + DEPTH] = emit_exchange(u + DEPTH, pend[u + DEPTH])
    return nc


def make_host_constants(T=4096, dtype_np=np.float32):
    """Host-precomputed decay constants (see module docstring)."""
    N = T // TBLK
    d = np.float64(DECAY)
    D128 = d ** TBLK
    s = np.arange(TBLK, dtype=np.float64)
    t = np.arange(TBLK, dtype=np.float64)
    nn = np.arange(N, dtype=np.float64)

    # masks[s, n, t] = 1[s<=t] * d^(-s-1) * D128^-(n-1)
    tri = (s[:, None] <= t[None, :]).astype(np.float64)  # [s, t]
    m = tri[:, None, :] * (d ** (-s - 1.0))[:, None, None] \
        * (D128 ** (-(nn - 1.0)))[None, :, None]
    masks = m.astype(BF16)

    # vscale[t, n] = d^(127-t) D128^-n  (applied to kn against v_u, which
    # already carries the (1-d) factor)
    vsc = (d ** (127.0 - t))[:, None] * (D128 ** (-nn))[None, :]
    vscale = vsc.astype(np.float32)

    # escale[t, n] = d^(t+1) D128^(n-1)
    esc = (d ** (t + 1.0))[:, None] * (D128 ** (nn - 1.0))[None, :]
    escale = esc.astype(np.float32)

    ident = np.eye(128, dtype=E4M3)
    return masks, vscale, escale, ident


_NC_CACHE = {}


def _get_nc(T, E, Dk, Dvh):
    key = (T, E, Dk, Dvh)
    if key not in _NC_CACHE:
        nc = build_nc(T=T, E=E, Dk=Dk, Dvh=Dvh)
        nc.finalize()
        _NC_CACHE[key] = nc
    return _NC_CACHE[key]


def kernel(x, Wv, Wk, bk, Wq, bq, Wo):
    y, _ = run(x, Wv, Wk, bk, Wq, bq, Wo)
    return y


def _install_ntff_hook():
    """The agent image's antenv lacks axon_hooks; recreate it from
    trn_boot's ctypes NTFF driver so trace=True produces profiles."""
    try:
        from antenv.axon_hooks import get_axon_ntff_profile_hook  # noqa: F401
        return
    except ImportError:
        pass
    try:
        import types
        import antenv
        from trn_agent_boot.trn_boot import _ntff_profile_via_ctypes
        hook = _ntff_profile_via_ctypes("/opt/axon/libaxon_pjrt.so")
        mod = types.ModuleType("antenv.axon_hooks")
        _h = {"hook": hook}
        mod.get_axon_ntff_profile_hook = lambda: _h["hook"]
        mod.set_axon_ntff_profile_hook = lambda h: _h.update(hook=h)
        sys.modules["antenv.axon_hooks"] = mod
        antenv.axon_hooks = mod
    except Exception as e:  # profiling is best-effort
        print(f"ntff hook install failed: {e}")


def _arrange_xt(xb, ST=512, dtype=BF16):
    """x[b] [T, E] -> xT pre-tiled [NU, 128, EC, ST], contiguous."""
    T, E = xb.shape
    xT = np.ascontiguousarray(xb.T).astype(dtype)         # [E, T]
    EC, NU = E // 128, T // ST
    return np.ascontiguousarray(
        xT.reshape(EC, 128, NU, ST).transpose(2, 1, 0, 3))


def _q8(a, scale=1.0):
    """Quantize to fp8 e4m3 (TRN-compatible range: clip +-240)."""
    return np.clip(np.asarray(a, np.float32) * scale, -240.0, 240.0).astype(E4M3)


def _arrange_w(w):
    """[E-or-Dv, D] -> [128, chunks, D] with row = chunk*128 + p."""
    R, D = w.shape
    C = R // 128
    return np.ascontiguousarray(w.reshape(C, 128, D).transpose(1, 0, 2))


def _arrange_b(b):
    b = np.asarray(b, np.float32).reshape(-1)
    C = b.shape[0] // 128
    return np.ascontiguousarray(b.reshape(C, 128).T)


def run(x, Wv, Wk, bk, Wq, bq, Wo, trace=False):
    x = np.asarray(x)
    B, T, E = x.shape
    Dk = np.asarray(Wk).shape[1]
    Dv = np.asarray(Wv).shape[1]
    Dvh = Dv // 2
    assert B == 4, "sharding is hardcoded for B=4 x 2 Dv-halves"

    nc = _get_nc(T, E, Dk, Dvh)
    masks, vscale, escale, ident = make_host_constants(T=T)

    wq_f8 = _q8(Wq, WSCALE)
    wk_f8 = _q8(Wk, WSCALE)
    bq32 = np.asarray(bq, np.float32).reshape(Dk, 1)
    bk32 = np.asarray(bk, np.float32).reshape(Dk, 1)
    Dkh = Dk // 2

    xt_cache = [_arrange_xt(x[b]) for b in range(B)]
    xt8_cache = [_arrange_xt(x[b], dtype=E4M3) for b in range(B)]
    in_maps = []
    for c in range(8):
        b, h = divmod(c, 2)
        dvs = slice(h * Dvh, (h + 1) * Dvh)
        # this core computes the q/k dk-half matching its pair rank
        dks = slice(h * Dkh, (h + 1) * Dkh)
        in_maps.append({
            "xt": xt_cache[b],
            "xt8": xt8_cache[b],
            "wq": _arrange_w(wq_f8[:, dks]),
            "wk": _arrange_w(wk_f8[:, dks]),
            "wv": _arrange_w(np.asarray(Wv[:, dvs], BF16)),
            "wo": _arrange_w(np.asarray(Wo[dvs], BF16)),
            "bq": _arrange_b(bq32[dks]),
            "bk": _arrange_b(bk32[dks]),
            "masks": masks,
            "vscale": vscale,
            "escale": escale,
            "ident": ident,
        })

    if trace:
        _install_ntff_hook()
    res = run_bass_kernel_spmd(nc, in_maps, core_ids=list(range(8)), trace=trace)
    y = np.zeros((B, T, E), np.float32)
    for c in range(8):
        b = c // 2
        y[b] += np.asarray(res.results[c]["out"], np.float32)
    return y, res



# revision 29
# speedup vs baseline: 1.2032x; 1.0226x over previous
"""Trainium2 Bass kernel for nn_Decay2DBlk (block-decay linear attention).

Full-input contract: kernel(**inputs) takes the unsharded inputs from
setup_inputs() and returns the full [B, T, E] output.

Sharding: 8 cores = 4 batch elements x 2 Dv-halves. Each core computes a
partial output y_b_h = (attn(x_b) restricted to its Dv half) @ Wo[half];
the host sums the two partials per batch element (the "all-reduce after
w_out" done host-side since outputs are gathered anyway).

Math (per core): with d=0.99, D=d^128, block index n, in-block offsets
s,t (keys/queries):
  out[t] = sum_{s<=t} d^(t_g - s_g) * q_t k_s * v_s  @ Wo      (t_g global)
All decay factors are folded into host-precomputed constants so the device
only does matmuls + one elementwise mask multiply + a running-sum state:
  - masks[n][s,t]  = 1[s<=t] d^(-s-1) D^-(n-1)        (intra-block, bf16)
  - vscale[n][t]   = (1-d) d^(127-t) D^-n             (v' for state update)
  - escale[n][t]   = d^(t+1) D^(n-1)                  (final ACT evac scale)
  - state S~ = sum_m k_m^T v'_m  (pure running sum, bf16 in SBUF)
The geometric growth of D^-n keeps every intermediate within fp32/bf16
range (max ~1e19) and makes bf16 rounding of the running sum benign
(validated: rel err ~4e-3 vs fp32 reference).
"""

import os
import sys

for _p in (
    "/root/.axon_site",
    "/root/.axon_site/_ro/trn_rl_repo",
    "/root/.axon_site/_ro/pypackages",
    "/opt/trn_rl_repo",
):
    if os.path.isdir(_p) and _p not in sys.path:
        sys.path.append(_p)

import numpy as np
import ml_dtypes
from contextlib import ExitStack

import concourse.bacc as bacc
import concourse.tile as tile
from concourse import mybir
from concourse.bass_utils import run_bass_kernel_spmd

DECAY = 0.99
TBLK = 128
BF16 = ml_dtypes.bfloat16
E4M3 = ml_dtypes.float8_e4m3fn
BF = mybir.dt.bfloat16
F8 = mybir.dt.float8e4
F32 = mybir.dt.float32
WSCALE = 64.0  # fp8 pre-scale for Wq/Wk (values ~N(0,0.02) -> sweet spot)
DR = mybir.MatmulPerfMode.DoubleRow


def build_nc(T=4096, E=1024, Dk=1024, Dvh=512, ST=512, pair_groups=None):
    """Build the per-core Bass program. Same program runs on all 8 cores
    (SPMD); only the input data differs.

    q/k phase-A work is split across the two cores of a pair: each core
    computes sigmoid(x @ Wq_half + b_half) for the dk-half whose weights it
    was GIVEN as input (wq input is [E, Dk/2]), then the halves are
    exchanged with a pairwise AllGather through DRAM bounce buffers. Core
    2b (group rank 0) always carries the low half, so the gathered layout
    is identical on both cores and the program stays SPMD-symmetric."""
    N = T // TBLK       # number of 128-token blocks
    NU = T // ST        # number of super-tiles
    JB = ST // TBLK     # blocks per super-tile
    EC = E // 128       # E chunks (contraction)
    KC = Dk // 128      # Dk chunks
    HC = KC // 2        # dk chunks computed locally (half)
    DC = Dvh // 128     # Dv-half chunks
    Dkh = Dk // 2
    if pair_groups is None:
        pair_groups = [[0, 1], [2, 3], [4, 5], [6, 7]]

    nc = bacc.Bacc(num_devices=8)
    # all inputs host-pre-arranged to [128-partition, ...contiguous] layout so
    # every DMA is 128 descriptors of 4-8KB (max descriptor efficiency)
    xt = nc.dram_tensor("xt", [NU, 128, EC, ST], BF, kind="ExternalInput")
    xt8 = nc.dram_tensor("xt8", [NU, 128, EC, ST], F8, kind="ExternalInput")
    wq = nc.dram_tensor("wq", [128, EC, Dkh], F8, kind="ExternalInput")
    wk = nc.dram_tensor("wk", [128, EC, Dkh], F8, kind="ExternalInput")
    wv = nc.dram_tensor("wv", [128, EC, Dvh], BF, kind="ExternalInput")
    wo = nc.dram_tensor("wo", [128, DC, E], BF, kind="ExternalInput")
    bq = nc.dram_tensor("bq", [128, HC], F32, kind="ExternalInput")
    bk = nc.dram_tensor("bk", [128, HC], F32, kind="ExternalInput")
    masks = nc.dram_tensor("masks", [128, N, 128], BF, kind="ExternalInput")
    vscale = nc.dram_tensor("vscale", [128, N], F32, kind="ExternalInput")
    escale = nc.dram_tensor("escale", [128, N], F32, kind="ExternalInput")
    ident = nc.dram_tensor("ident", [128, 128], F8, kind="ExternalInput")
    out = nc.dram_tensor("out", [T, E], BF, kind="ExternalOutput")

    SIG = mybir.ActivationFunctionType.Sigmoid
    CPY = mybir.ActivationFunctionType.Copy

    with tile.TileContext(nc) as tc:
        with ExitStack() as ctx:
            consts = ctx.enter_context(tc.tile_pool(name="consts", bufs=1))
            qkh_pool = ctx.enter_context(tc.tile_pool(name="qkh", bufs=2))
            dram = ctx.enter_context(tc.tile_pool(name="dram", bufs=6, space="DRAM"))
            xt_pool = ctx.enter_context(tc.tile_pool(name="xt", bufs=3))
            xt8_pool = ctx.enter_context(tc.tile_pool(name="xt8", bufs=3))
            qk_pool = ctx.enter_context(tc.tile_pool(name="qk", bufs=3))
            v_pool = ctx.enter_context(tc.tile_pool(name="v", bufs=6))
            kn_pool = ctx.enter_context(tc.tile_pool(name="kn", bufs=3))
            ap_pool = ctx.enter_context(tc.tile_pool(name="apool", bufs=3))
            yt_pool = ctx.enter_context(tc.tile_pool(name="yt", bufs=3))
            o_pool = ctx.enter_context(tc.tile_pool(name="opool", bufs=2))
            st_pool = ctx.enter_context(tc.tile_pool(name="state", bufs=1))
            psA = ctx.enter_context(tc.tile_pool(name="psA", bufs=2, space="PSUM"))
            psB = ctx.enter_context(tc.tile_pool(name="psB", bufs=4, space="PSUM"))
            psY = ctx.enter_context(tc.tile_pool(name="psY", bufs=2, space="PSUM"))

            # ---- constants into SBUF ----
            # startup-latency critical: the very first matmul needs only
            # wq8 + bq + xt8[0] (1.25MB total in fp8) -- issue those first;
            # everything else streams in under cover of the fp8 q/k matmuls
            wq_sb = consts.tile([128, EC, Dkh], F8)
            nc.sync.dma_start(out=wq_sb, in_=wq[:, :, :])
            bq_sb = consts.tile([128, HC], F32)
            nc.sync.dma_start(out=bq_sb, in_=bq[:, :])
            wk_sb = consts.tile([128, EC, Dkh], F8)
            nc.sync.dma_start(out=wk_sb, in_=wk[:, :, :])
            bk_sb = consts.tile([128, HC], F32)
            nc.sync.dma_start(out=bk_sb, in_=bk[:, :])
            # deferred const tiles (DMAs issued inside/after phase A 0)
            vs_sb = consts.tile([128, N], F32)
            es_sb = consts.tile([128, N], F32)
            id_sb = consts.tile([128, 128], F8)
            wv_sb = consts.tile([128, EC, Dvh], BF)
            mk_sb = consts.tile([128, N, 128], BF)
            wo_sb = consts.tile([128, DC, E], BF)

            def emit_deferred_consts():
                nc.sync.dma_start(out=wv_sb, in_=wv[:, :, :])
                nc.sync.dma_start(out=vs_sb, in_=vscale[:, :])
                nc.sync.dma_start(out=es_sb, in_=escale[:, :])
                nc.sync.dma_start(out=id_sb, in_=ident[:, :])

            # persistent scaled-sum state S~ [dk, dv], one tile per dk-chunk
            # (separate tiles -> per-chunk dependency chains, so the y2 reads
            # of block n+1 pipeline against the state adds of block n)
            S_c = [st_pool.tile([128, Dvh], BF, name=f"S{c}", tag=f"S{c}")
                   for c in range(KC)]

            def emit_phase_a(u, first=False):
                # ---- load xT super-tile: fp8 copy first (feeds q/k
                # DoubleRow matmuls immediately), bf16 behind it (for v) ----
                xt8_u = xt8_pool.tile([128, EC, ST], F8, name="xt8_u")
                if first:
                    nc.sync.dma_start(out=xt8_u[:, :EC // 2], in_=xt8[u, :, :EC // 2])
                    nc.sync.dma_start(out=xt8_u[:, EC // 2:], in_=xt8[u, :, EC // 2:])
                else:
                    nc.sync.dma_start(out=xt8_u, in_=xt8[u])
                xt_u = xt_pool.tile([128, EC, ST], BF, name="xt_u")
                nc.sync.dma_start(out=xt_u, in_=xt[u])
                if first:
                    emit_deferred_consts()

                # ---- phase A: this core computes its dk-half of qT, kT ----
                # fp8 DoubleRow: contraction pairs of e-chunks, 2x PE rate.
                # q and k halves live in ONE tile [128, 2(qk), HC, ST] so the
                # exchange staging is a single DMA trigger.
                qkh_u = qkh_pool.tile([128, 2, HC, ST], F8, name="qkh_u", tag="qkh")
                for qk, (w_sb, b_sb) in enumerate(((wq_sb, bq_sb), (wk_sb, bk_sb))):
                    for c in range(HC):
                        ps = psA.tile([128, ST], F32, name="psqk", tag="psA")
                        for e in range(EC // 2):
                            nc.tensor.matmul(
                                ps, w_sb[:, 2 * e:2 * e + 2, c * 128:(c + 1) * 128],
                                xt8_u[:, 2 * e:2 * e + 2, :],
                                start=(e == 0), stop=(e == EC // 2 - 1), perf_mode=DR)
                        nc.scalar.activation(qkh_u[:, qk, c, :], ps, SIG,
                                             bias=b_sb[:, c:c + 1], scale=1.0 / WSCALE)

                v_u = v_pool.tile([128, JB, Dvh], BF, name="v_u", tag="v")
                for j in range(JB):
                    ps = psA.tile([128, ST], F32, name="psv", tag="psA")[:, :Dvh]
                    for e in range(EC):
                        nc.tensor.matmul(
                            ps, xt_u[:, e, j * 128:(j + 1) * 128], wv_sb[:, e, :],
                            start=(e == 0), stop=(e == EC - 1))
                    nc.scalar.activation(v_u[:, j, :], ps, CPY, scale=1.0 - DECAY)
                return qkh_u, v_u

            def emit_exchange(u, ph):
                # pairwise AllGather of the q/k dk-halves through DRAM bounce
                # buffers; staging DMAs ride the sync HWDGE ring (fast static
                # descriptors) -- only the collective itself is on CC
                qkh_u, _ = ph
                bin_u = dram.tile([128, 2, HC, ST], F8, name="bin_u", tag="bin")
                nc.scalar.dma_start(out=bin_u, in_=qkh_u)
                bout_u = dram.tile([2, 128, 2, HC, ST], F8, name="bout_u", tag="bout")
                nc.gpsimd.collective_compute(
                    "AllGather", mybir.AluOpType.bypass,
                    replica_groups=pair_groups,
                    ins=[bin_u.opt()], outs=[bout_u.opt()])
                return bout_u

            def emit_gather(u, bout_u):
                # one DMA: [g, p, qk, h, t] -> qk_u[p, qk, g*HC+h, t]
                qk_u = qk_pool.tile([128, 2, KC, ST], F8, name="qk_u", tag="qk")
                nc.gpsimd.dma_start(
                    out=qk_u, in_=bout_u.rearrange("g p q h t -> p q g h t"))
                return qk_u

            def emit_blocks(u, tiles, qk_u):
                _, v_u = tiles
                qT_u = qk_u[:, 0]
                kT_u = qk_u[:, 1]
                # ---- block loop ----
                for j in range(JB):
                    n = u * JB + j
                    jsl = slice(j * 128, (j + 1) * 128)

                    # intra-block attention logits a^T[s,t], masked
                    a_ps = psY.tile([128, 512], F32, name="a_ps", tag="psY")[:, :128]
                    for c in range(KC):
                        nc.tensor.matmul(
                            a_ps, kT_u[:, c, jsl], qT_u[:, c, jsl],
                            start=(c == 0), stop=(c == KC - 1))
                    a_sb = ap_pool.tile([128, 128], BF, name="a_sb")
                    nc.vector.tensor_mul(a_sb, a_ps, mk_sb[:, n, :])

                    # k natural [s, dk] via PE transposes of kT, scaled by
                    # vscale[s,n] during the PSUM->SBUF copy (so the kv state
                    # matmul can consume plain v_u -- no separate vp tensor)
                    kn = kn_pool.tile([128, Dk], BF, name="kn")
                    for h in range(KC // 4):
                        # fp8 PE transpose writes with element step 2 (hw
                        # requirement), so each 128-col result spans 256
                        # elements; the strided AP is collapsed by the copy
                        tp = psB.tile([128, 1024], F8, name="tp", tag="psB")[:, 0:1024:2]
                        for q4 in range(4):
                            c = h * 4 + q4
                            nc.tensor.transpose(
                                tp[:, q4 * 128:(q4 + 1) * 128],
                                kT_u[:, c, jsl], id_sb)
                        if h == 0:
                            nc.scalar.activation(
                                kn[:, h * 512:(h + 1) * 512], tp, CPY,
                                scale=vs_sb[:, n:n + 1])
                        else:
                            nc.scalar.activation(
                                kn[:, h * 512:(h + 1) * 512], tp, CPY,
                                scale=vs_sb[:, n:n + 1])

                    # yT[dv, t] = v^T a' + S~^T-contract (cross), unscaled
                    y_ps = psY.tile([128, 4 * 128], F32, name="y_ps", tag="psY")
                    for dc in range(DC):
                        osl = slice(dc * 128, (dc + 1) * 128)
                        dvsl = slice(dc * 128, (dc + 1) * 128)
                        nc.tensor.matmul(
                            y_ps[:, osl], v_u[:, j, dvsl], a_sb,
                            start=True, stop=(n == 0))
                        if n > 0:
                            for c in range(KC):
                                nc.tensor.matmul(
                                    y_ps[:, osl], S_c[c][:, dvsl], qT_u[:, c, jsl],
                                    start=False, stop=(c == KC - 1))
                    yT_sb = yt_pool.tile([128, 4 * 128], BF, name="yT_sb")
                    nc.scalar.copy(yT_sb, y_ps)

                    # state update S~ += k'^T v (DVE adds trail the kv
                    # matmuls) interleaved with the out projection so the
                    # PSUM bank recycling (freed by each DVE add) always has
                    # a few hundred ns of PE work in front of it
                    o_sb = o_pool.tile([128, E], BF, name="o_sb")

                    def emit_kv(c):
                        kv_ps = psB.tile([128, 512], F32, name="kv_ps", tag="psB")[:, :Dvh]
                        nc.tensor.matmul(
                            kv_ps, kn[:, c * 128:(c + 1) * 128], v_u[:, j, :],
                            start=True, stop=True)
                        if n == 0:
                            nc.vector.tensor_copy(S_c[c], kv_ps)
                        else:
                            nc.vector.tensor_add(S_c[c], S_c[c], kv_ps)

                    def emit_out(hh):
                        # out[t, e-half] = yT^T @ Wo, evacuated with escale
                        o_ps = psB.tile([128, 512], F32, name="o_ps", tag="psB")
                        for dc in range(DC):
                            nc.tensor.matmul(
                                o_ps, yT_sb[:, dc * 128:(dc + 1) * 128],
                                wo_sb[:, dc, hh * 512:(hh + 1) * 512],
                                start=(dc == 0), stop=(dc == DC - 1))
                        nc.scalar.activation(
                            o_sb[:, hh * 512:(hh + 1) * 512], o_ps, CPY,
                            scale=es_sb[:, n:n + 1])

                    for c in range(4):
                        emit_kv(c)
                    emit_out(0)
                    for c in range(4, KC):
                        emit_kv(c)
                    emit_out(1)
                    # out store rides the scalar HWDGE ring: its dependency
                    # (the o_sb evac, also on scalar) is met by queue order,
                    # so it never head-of-line-blocks anything
                    nc.scalar.dma_start(
                        out=out[n * 128:(n + 1) * 128, :], in_=o_sb)

            # ---- PE warmup: dummy matmuls on never-written SBUF keep the
            # tensor engine busy from t~0 so the HAM clock-gate reaches
            # K=8/8 (2.4GHz) before the first real matmul, instead of
            # running phase A at 1.2GHz ----
            junk = consts.tile([128, 128], BF, name="junk")
            nc.vector.memset(junk, 0)
            wps = psB.tile([128, 512], F32, name="warm", tag="psB")
            for _ in range(44):
                nc.tensor.matmul(wps[:, :128], junk, junk, start=True, stop=True)

            # Software pipeline, depth 5: the AllGather control plane has
            # a ~40us latency floor per call, so the first 5 phase A's are
            # issued up front -- the PE chews through them while the first
            # exchanges complete, and every gather lands with slack.
            DEPTH = 5
            pend = {}
            xch = {}
            for uu in range(min(DEPTH, NU)):
                pend[uu] = emit_phase_a(uu, first=(uu == 0))
                xch[uu] = emit_exchange(uu, pend[uu])
                if uu == 2:
                    nc.sync.dma_start(out=mk_sb, in_=masks[:, :, :])
                    nc.sync.dma_start(out=wo_sb, in_=wo[:, :, :])
            for u in range(NU):
                qkt = emit_gather(u, xch.pop(u))
                emit_blocks(u, pend.pop(u), qkt)
                if u + DEPTH < NU:
                    pend[u + DEPTH] = emit_phase_a(u + DEPTH)
                    xch[u + DEPTH] = emit_exchange(u + DEPTH, pend[u + DEPTH])
    return nc


def make_host_constants(T=4096, dtype_np=np.float32):
    """Host-precomputed decay constants (see module docstring)."""
    N = T // TBLK
    d = np.float64(DECAY)
    D128 = d ** TBLK
    s = np.arange(TBLK, dtype=np.float64)
    t = np.arange(TBLK, dtype=np.float64)
    nn = np.arange(N, dtype=np.float64)

    # masks[s, n, t] = 1[s<=t] * d^(-s-1) * D128^-(n-1)
    tri = (s[:, None] <= t[None, :]).astype(np.float64)  # [s, t]
    m = tri[:, None, :] * (d ** (-s - 1.0))[:, None, None] \
        * (D128 ** (-(nn - 1.0)))[None, :, None]
    masks = m.astype(BF16)

    # vscale[t, n] = d^(127-t) D128^-n  (applied to kn against v_u, which
    # already carries the (1-d) factor)
    vsc = (d ** (127.0 - t))[:, None] * (D128 ** (-nn))[None, :]
    vscale = vsc.astype(np.float32)

    # escale[t, n] = d^(t+1) D128^(n-1)
    esc = (d ** (t + 1.0))[:, None] * (D128 ** (nn - 1.0))[None, :]
    escale = esc.astype(np.float32)

    ident = np.eye(128, dtype=E4M3)
    return masks, vscale, escale, ident


_NC_CACHE = {}


def _get_nc(T, E, Dk, Dvh):
    key = (T, E, Dk, Dvh)
    if key not in _NC_CACHE:
        nc = build_nc(T=T, E=E, Dk=Dk, Dvh=Dvh)
        nc.finalize()
        _NC_CACHE[key] = nc
    return _NC_CACHE[key]


def kernel(x, Wv, Wk, bk, Wq, bq, Wo):
    y, _ = run(x, Wv, Wk, bk, Wq, bq, Wo)
    return y


def _install_ntff_hook():
    """The agent image's antenv lacks axon_hooks; recreate it from
    trn_boot's ctypes NTFF driver so trace=True produces profiles."""
    try:
        from antenv.axon_hooks import get_axon_ntff_profile_hook  # noqa: F401
        return
    except ImportError:
        pass
    try:
        import types
        import antenv
        from trn_agent_boot.trn_boot import _ntff_profile_via_ctypes
        hook = _ntff_profile_via_ctypes("/opt/axon/libaxon_pjrt.so")
        mod = types.ModuleType("antenv.axon_hooks")
        _h = {"hook": hook}
        mod.get_axon_ntff_profile_hook = lambda: _h["hook"]
        mod.set_axon_ntff_profile_hook = lambda h: _h.update(hook=h)
        sys.modules["antenv.axon_hooks"] = mod
        antenv.axon_hooks = mod
    except Exception as e:  # profiling is best-effort
        print(f"ntff hook install failed: {e}")


def _arrange_xt(xb, ST=512, dtype=BF16):
    """x[b] [T, E] -> xT pre-tiled [NU, 128, EC, ST], contiguous."""
    T, E = xb.shape
    xT = np.ascontiguousarray(xb.T).astype(dtype)         # [E, T]
    EC, NU = E // 128, T // ST
    return np.ascontiguousarray(
        xT.reshape(EC, 128, NU, ST).transpose(2, 1, 0, 3))


def _q8(a, scale=1.0):
    """Quantize to fp8 e4m3 (TRN-compatible range: clip +-240)."""
    return np.clip(np.asarray(a, np.float32) * scale, -240.0, 240.0).astype(E4M3)


def _arrange_w(w):
    """[E-or-Dv, D] -> [128, chunks, D] with row = chunk*128 + p."""
    R, D = w.shape
    C = R // 128
    return np.ascontiguousarray(w.reshape(C, 128, D).transpose(1, 0, 2))


def _arrange_b(b):
    b = np.asarray(b, np.float32).reshape(-1)
    C = b.shape[0] // 128
    return np.ascontiguousarray(b.reshape(C, 128).T)


def run(x, Wv, Wk, bk, Wq, bq, Wo, trace=False):
    x = np.asarray(x)
    B, T, E = x.shape
    Dk = np.asarray(Wk).shape[1]
    Dv = np.asarray(Wv).shape[1]
    Dvh = Dv // 2
    assert B == 4, "sharding is hardcoded for B=4 x 2 Dv-halves"

    nc = _get_nc(T, E, Dk, Dvh)
    masks, vscale, escale, ident = make_host_constants(T=T)

    wq_f8 = _q8(Wq, WSCALE)
    wk_f8 = _q8(Wk, WSCALE)
    bq32 = np.asarray(bq, np.float32).reshape(Dk, 1)
    bk32 = np.asarray(bk, np.float32).reshape(Dk, 1)
    Dkh = Dk // 2

    xt_cache = [_arrange_xt(x[b]) for b in range(B)]
    xt8_cache = [_arrange_xt(x[b], dtype=E4M3) for b in range(B)]
    in_maps = []
    for c in range(8):
        b, h = divmod(c, 2)
        dvs = slice(h * Dvh, (h + 1) * Dvh)
        # this core computes the q/k dk-half matching its pair rank
        dks = slice(h * Dkh, (h + 1) * Dkh)
        in_maps.append({
            "xt": xt_cache[b],
            "xt8": xt8_cache[b],
            "wq": _arrange_w(wq_f8[:, dks]),
            "wk": _arrange_w(wk_f8[:, dks]),
            "wv": _arrange_w(np.asarray(Wv[:, dvs], BF16)),
            "wo": _arrange_w(np.asarray(Wo[dvs], BF16)),
            "bq": _arrange_b(bq32[dks]),
            "bk": _arrange_b(bk32[dks]),
            "masks": masks,
            "vscale": vscale,
            "escale": escale,
            "ident": ident,
        })

    if trace:
        _install_ntff_hook()
    res = run_bass_kernel_spmd(nc, in_maps, core_ids=list(range(8)), trace=trace)
    y = np.zeros((B, T, E), np.float32)
    for c in range(8):
        b = c // 2
        y[b] += np.asarray(res.results[c]["out"], np.float32)
    return y, res

